# revision 1
# baseline (speedup 1.0000x reference)
"""Trainium2 Bass kernel for sparse (strided) multi-head attention.

Reference computation (B=2, S=2048, H=1024, NH=16, D=64):
    q = (x @ q_w) * sigmoid(phi); k = x @ k_w; v = x @ v_w   (per-head [S, D])
    scores = q k^T / sqrt(D), masked to allowed[i, j] = (j % 4 == 0) | (|i-j| <= 8)
    out = softmax(scores) @ v;  return concat_heads(out) @ o_w + o_b

Sharding: 8 cores = 2 batches x 4 head-groups (4 heads each). Each core gets
x^T for its batch, column-sliced q/k/v weights, row-sliced o_w, and returns a
partial transposed output F^T = (attn_out_heads @ o_w_slice)^T which the host
sums over head-groups, transposes, and biases.

v2 design notes (all matmuls bf16; PSUM stays f32):
  - The local +/-8 band is handled with 19 query-chunks of 112 queries, each
    against a 128-key window starting 8 keys before the chunk (chunk 0 is
    key-aligned, chunk 18 is the 32-query tail against 40 keys). One window
    fully covers every query's band, so there are no corner pieces. The
    multiplicative 0/1 post-exp masks are identical for all middle chunks.
  - V is produced directly in the shifted-window layout (sb_Vsh) by running
    the V projection with shifted x^T column windows; the strided-key V
    (sb_Vs) is produced by projecting x^T[:, ::4] via a stride-4 lhsT access
    pattern. No SBUF->SBUF gather DMAs.
  - DMA count is minimized (~19/iter vs ~119 in v1): each HWDGE DMA costs
    ~650ns sequencer + 625ns shared-HWDGE + 900ns semaphore propagation.
    Ones columns come from gpsimd memset, masks ship in one tensor, the
    softmax-denominator broadcast uses gpsimd partition_broadcast (SWDGE)
    instead of a DRAM bounce, and output DMAs are batched per 128-row tile.
  - Engine budget: PE ~95us (projections + scores + attn@V), ACT ~42us
    (exps; the phase-2 co-bottleneck), DVE (PSUM copies, reciprocals,
    normalize muls), Pool (mask muls, broadcast, memsets; cannot touch PSUM).
"""

import os
import numpy as np

B, S, H = 2, 2048, 1024
NH, D = 16, 64
PHI = 1.6180339887
STRIDE, LOCAL = 4, 8
HPG = 4              # heads per group (= per core)
GC = HPG * D         # channels per core = 256
NSK = S // STRIDE    # 512 strided keys
QCW = 112            # band query-chunk width
NQC = 19             # number of band chunks (18*112 + 32 = 2048)

_CACHE = {}
LAST_RESULTS = None  # BassKernelResults of the most recent run (for profiling)


def _chunk_geom(c):
    """(key-window start, window width, query base, query count) for chunk c."""
    if c == 0:
        return 0, 128, 0, QCW
    if c == NQC - 1:
        return 2008, 40, 2016, 32
    return QCW * c - 8, 128, QCW * c, QCW


def host_masks():
    """Multiplicative 0/1 masks applied to exp'd band scores.

    mask0 is for the key-aligned chunk 0 (key kp, query q):
        |q - kp| <= 8 and kp % 4 != 0
    maskN is for the shifted chunks c >= 1 (key 112c-8+kp):
        kp-16 <= q <= kp and kp % 4 != 0
    """
    kp = np.arange(128)[:, None]
    q = np.arange(QCW)[None, :]
    mask0 = ((np.abs(q - kp) <= LOCAL) & (kp % STRIDE != 0)).astype(np.float32)
    maskN = ((q >= kp - 2 * LOCAL) & (q <= kp) & (kp % STRIDE != 0)).astype(np.float32)
    return np.concatenate([mask0, maskN], axis=1)  # [128, 224]


def _dtypes():
    import concourse.mybir as mybir
    name = os.environ.get("KERNEL_MM_DTYPE", "bfloat16")
    dt = {"float32": mybir.dt.float32, "float32r": mybir.dt.float32r,
          "bfloat16": mybir.dt.bfloat16}[name]
    out_dt = (mybir.dt.float32 if os.environ.get("KERNEL_OUT_F32")
              else mybir.dt.bfloat16)
    return dt, out_dt


def build_nc(loop_n=1):
    """Build the per-core Bass program (same NEFF for all 8 cores).

    loop_n > 1 wraps the whole pipeline in a hardware loop (benchmarking:
    wall-clock deltas between loop counts cancel dispatch overhead).

    Emission order is engine program order, so the stream is scheduled
    explicitly for overlap:
      Phase A: input DMAs, Q/K ct0 projections, then heads 1 and 0's score
        matmuls interleaved with the V projections -- the ACT engine chews
        exps while PE fills with projection work.
      Phase B: attn@V + normalize chains for heads 1,0; ct=1 Q/K projection
        waves as PE filler; then heads 3,2 scores/attn@V/chains.
      Phase C: output projection F^T, batched DMA out per 128-row tile.
    """
    import contextlib
    import concourse.mybir as mybir
    import concourse.tile as tile
    from concourse import bacc

    f32 = mybir.dt.float32
    DT, OUT_DT = _dtypes()
    AF = mybir.ActivationFunctionType

    nc = bacc.Bacc("TRN2", target_bir_lowering=False, debug=False)

    XW = S + 3 * GC + 2 * QCW  # x^T | q/k/v weights | band masks (t=0 rows)
    d_xin = nc.dram_tensor("xin", [H, XW], DT, kind="ExternalInput")
    d_ow = nc.dram_tensor("ow", [GC, H], DT, kind="ExternalInput")
    d_fT = nc.dram_tensor("fT", [H, S], OUT_DT, kind="ExternalOutput")

    def mm(out, lhsT, rhs, start, stop):
        nc.tensor.matmul(out, lhsT, rhs, start=start, stop=stop,
                         skip_group_check=True)

    with tile.TileContext(nc) as tc:
        with (
            tc.tile_pool(name="consts", bufs=1) as consts,
            tc.tile_pool(name="persist", bufs=1) as persist,
            tc.tile_pool(name="ph1", bufs=1) as ph1,
        ):
            sb_ow = persist.tile([128, 2, 1024], DT)

            # D-major Q^T / K^T: [128ch (2 heads), c-tile, S]
            sb_QT = persist.tile([128, 2, S], DT)
            sb_KT = persist.tile([128, 2, S], DT)
            sb_KsT = persist.tile([128, 2, NSK], DT)      # strided keys, compacted
            # S-major V, shifted band windows + strided keys; col 64 = 1.0
            sb_Vsh = persist.tile([128, NQC, HPG, 66], DT)
            sb_Vs = persist.tile([128, NSK // 128, HPG, 66], DT)
            sb_outTs = persist.tile([128, 2, S], DT)      # c-major head outputs

            sb_xin = ph1.tile([128, 8, XW], DT)
            sb_xT = sb_xin[:, :, 0:S]
            sb_qw = sb_xin[:, :, S:S + GC]
            sb_kw = sb_xin[:, :, S + GC:S + 2 * GC]
            sb_vw = sb_xin[:, :, S + 2 * GC:S + 3 * GC]
            sb_maskb = sb_xin[:, 0, S + 3 * GC:].rearrange(
                "p (m q) -> p m q", m=2)

            loop_cm = tc.For_i(0, loop_n, 1) if loop_n > 1 else contextlib.nullcontext()
            with loop_cm, (
                tc.tile_pool(name="ats", bufs=12)) as p_ats, (
                tc.tile_pool(name="atb", bufs=4)) as p_atb, (
                tc.tile_pool(name="rec", bufs=2)) as p_rec, (
                tc.tile_pool(name="bc", bufs=2)) as p_bc, (
                tc.tile_pool(name="ost", bufs=2)) as p_ost, (
                tc.tile_pool(name="stage", bufs=2)) as p_stage, (
                tc.tile_pool(name="psS", bufs=2, space="PSUM")) as psS, (
                tc.tile_pool(name="psB", bufs=6, space="PSUM")) as psB:

                # ---------------- shared emit helpers ----------------
                def hslices(h):
                    ct, pb = h // 2, (h % 2) * 64
                    return (ct, pb, sb_QT[pb:pb + 64, ct, :],
                            sb_KT[pb:pb + 64, ct, :], sb_KsT[pb:pb + 64, ct, :])

                def sc_strided(h, b, at_s):
                    """Strided scores for query block b (all 4 key tiles), so
                    attn@V block b is ready after 4 exps instead of a full
                    head's worth."""
                    ct, pb, QT, KT, KsT = hslices(h)
                    ql = slice(512 * b, 512 * (b + 1))
                    for i in range(4):
                        ps = psS.tile([128, 512], f32, tag="sc", name="ps_sc")
                        mm(ps[:], KsT[:, 128 * i:128 * (i + 1)], QT[:, ql],
                           start=True, stop=True)
                        nc.scalar.activation(at_s[i][:, ql], ps[:], AF.Exp)

                def band_group(h, g, at_b):
                    """Band-score psum group g: matmuls + exp + 0/1 mask."""
                    ct, pb, QT, KT, _ = hslices(h)
                    chunks = list(range(4 * g, min(4 * g + 4, NQC)))
                    ps = psS.tile([128, 448], f32, tag="sc", name="ps_bd")
                    for j, c in enumerate(chunks):
                        w0, kw_, qb, nq = _chunk_geom(c)
                        mm(ps[0:kw_, 112 * j:112 * j + nq],
                           KT[:, w0:w0 + kw_], QT[:, qb:qb + nq],
                           start=True, stop=True)
                    if g < 4:
                        nc.scalar.activation(at_b[:, 448 * g:448 * (g + 1)],
                                             ps[:], AF.Exp)
                    else:
                        # rows 40:128 of the chunk-18 columns are stale psum
                        # exp'd to garbage; never read (attn@V and the mask
                        # touch only [0:40] there)
                        nc.scalar.activation(at_b[:, 1792:2048], ps[:, 0:256],
                                             AF.Exp)
                    mN = sb_maskb[:, 1, :]
                    if g == 0:
                        nc.gpsimd.tensor_mul(at_b[:, 0:112], at_b[:, 0:112],
                                             sb_maskb[:, 0, :])
                        sl = at_b[:, 112:448].rearrange("p (a b) -> p a b", b=112)
                        nc.gpsimd.tensor_mul(sl, sl, _free_bcast(mN, 3))
                    elif g < 4:
                        sl = at_b[:, 448 * g:448 * (g + 1)].rearrange(
                            "p (a b) -> p a b", b=112)
                        nc.gpsimd.tensor_mul(sl, sl, _free_bcast(mN, 4))
                    else:
                        sl = at_b[:, 1792:2016].rearrange("p (a b) -> p a b", b=112)
                        nc.gpsimd.tensor_mul(sl, sl, _free_bcast(mN, 2))
                        nc.gpsimd.tensor_mul(at_b[0:40, 2016:2048],
                                             at_b[0:40, 2016:2048],
                                             mN[0:40, 0:32])

                def emit_scores(h, at_s, at_b, filler):
                    """All scores for head h; filler() emits PE work between
                    waves (V projections in phase A, ct=1 Q/K in phase B)."""
                    for b in range(4):
                        sc_strided(h, b, at_s)
                        filler()
                    for g in range(5):
                        band_group(h, g, at_b)
                        if g % 2 == 1:
                            filler()

                def av_blocks(h, at_s, at_b, ost):
                    """attn@[V|1] blocks + batched normalize for head h.

                    Each 512-query block's psum holds values rows 0:64 and
                    the denominator row 64; reciprocals (bf16) collect into
                    one rec row per head, one partition_broadcast per head
                    (SWDGE ops carry a large fixed cost), then four muls.
                    pb=0 heads write sb_outTs directly; pb=64 heads stage
                    into their half of the shared ost tile (one repartition
                    DMA for both, issued from head 3's finisher).
                    """
                    ct, pb, _, _, _ = hslices(h)
                    rec = p_rec.tile([1, S], DT, tag="rec", name="rec")
                    bc = p_bc.tile([64, S], DT, tag="bc", name="bc")
                    pos = [None] * 4

                    def block(blk):
                        base = 512 * blk
                        po = psB.tile([65, 512], f32, tag="b", name="ps_out")
                        pos[blk] = po
                        for i in range(4):
                            mm(po[:], sb_Vs[:, i, h, 0:65],
                               at_s[i][:, base:base + 512],
                               start=(i == 0), stop=False)
                        last = []
                        for c in range(NQC):
                            _, kw_, qb, nq = _chunk_geom(c)
                            lo = max(qb, base)
                            hi = min(qb + nq, base + 512)
                            if lo < hi:
                                last.append((c, kw_, lo, hi))
                        for oi, (c, kw_, lo, hi) in enumerate(last):
                            mm(po[:, lo - base:hi - base],
                               sb_Vsh[0:kw_, c, h, 0:65],
                               at_b[0:kw_, lo:hi],
                               start=False, stop=(oi == len(last) - 1))
                        with nc.allow_low_precision("bf16 softmax denominators"):
                            nc.vector.reciprocal(rec[:, base:base + 512],
                                                 po[64:65, :])

                    def fin():
                        nc.gpsimd.partition_broadcast(bc[:], rec[:])
                        for blk in range(4):
                            ql = slice(512 * blk, 512 * (blk + 1))
                            dst = (sb_outTs[0:64, ct, ql] if pb == 0
                                   else ost[:, ct, ql])
                            nc.vector.tensor_mul(dst, pos[blk][0:64, :], bc[:, ql])
                        if pb and ct == 1:
                            # heads 1 and 3 staged; ship both halves at once
                            nc.sync.dma_start(out=sb_outTs[64:128, :, :],
                                              in_=ost[:])
                    return [lambda b=b: block(b) for b in range(4)] + [fin]

                # ---------------- Phase A ----------------
                if True:
                    # Each DMA carries a large fixed cost on hardware, so
                    # everything ships in the fewest possible transfers; in
                    # the benchmark loop the x^T load overlaps the previous
                    # iteration's output phase.
                    xin_r = d_xin.rearrange("(t p) s -> p t s", p=128)
                    nc.sync.dma_start(out=sb_xin[:], in_=xin_r[:])
                    nc.sync.dma_start(out=sb_ow[:], in_=d_ow.rearrange("(t p) f -> p t f", p=128))

                    # ones columns for the attn@V row-sum trick
                    nc.gpsimd.memset(sb_Vsh[:, :, :, 64], 1.0)
                    nc.gpsimd.memset(sb_Vs[:, :, :, 64], 1.0)

                    def qk_wave(w_sb, w_out, ct):
                        ps = [psB.tile([128, 512], f32, tag="b", name="psproj")
                              for _ in range(4)]
                        for ht in range(8):
                            for ss in range(4):
                                mm(ps[ss][:], w_sb[:, ht, 128 * ct:128 * (ct + 1)],
                                   sb_xT[:, ht, 512 * ss:512 * (ss + 1)],
                                   start=(ht == 0), stop=(ht == 7))
                        for ss in range(4):
                            nc.vector.tensor_copy(
                                w_out[:, ct, 512 * ss:512 * (ss + 1)], ps[ss][:])

                    def ks_compact(ct):
                        ks = sb_KT[:, ct, :].rearrange("p (r f) -> p r f", f=STRIDE)[:, :, 0]
                        nc.vector.tensor_copy(sb_KsT[:, ct, :], ks)

                    def qk0_wave():
                        # Q and K ct0 interleaved per-ht so PE consumption of
                        # the x^T pieces matches their DMA delivery rate; K
                        # accumulates in the two psS slots to leave psB for Q
                        psq = [psB.tile([128, 512], f32, tag="b", name="psproj")
                               for _ in range(4)]
                        psk = [psB.tile([128, 512], f32, tag="b", name="psprojk")
                               for _ in range(2)]
                        psk += [psS.tile([128, 512], f32, tag="sc", name="psprojk")
                                for _ in range(2)]
                        for ht in range(8):
                            for ss in range(4):
                                mm(psq[ss][:], sb_qw[:, ht, 0:128],
                                   sb_xT[:, ht, 512 * ss:512 * (ss + 1)],
                                   start=(ht == 0), stop=(ht == 7))
                            for ss in range(4):
                                mm(psk[ss][:], sb_kw[:, ht, 0:128],
                                   sb_xT[:, ht, 512 * ss:512 * (ss + 1)],
                                   start=(ht == 0), stop=(ht == 7))
                        for ss in range(4):
                            nc.vector.tensor_copy(
                                sb_QT[:, 0, 512 * ss:512 * (ss + 1)], psq[ss][:])
                        for ss in range(4):
                            nc.vector.tensor_copy(
                                sb_KT[:, 0, 512 * ss:512 * (ss + 1)], psk[ss][:])

                    qk0_wave()
                    ks_compact(0)

                    # V projection fillers: 5 shifted-window groups + strided
                    def v_group(g):
                        chunks = range(4 * g, min(4 * g + 4, NQC))
                        ps = [psB.tile([128, GC], f32, tag="b", name="psprojv")
                              for _ in chunks]
                        for ht in range(8):
                            for j, c in enumerate(chunks):
                                w0, kw_, _, _ = _chunk_geom(c)
                                mm(ps[j][0:kw_, :], sb_xT[:, ht, w0:w0 + kw_],
                                   sb_vw[:, ht, :], start=(ht == 0), stop=(ht == 7))
                        for j, c in enumerate(chunks):
                            _, kw_, _, _ = _chunk_geom(c)
                            nc.vector.tensor_copy(
                                sb_Vsh[0:kw_, c, :, 0:64],
                                ps[j][0:kw_, :].rearrange("p (h d) -> p h d", h=HPG))

                    def v_strided():
                        ps = [psB.tile([128, GC], f32, tag="b", name="psprojs")
                              for _ in range(4)]
                        for ht in range(8):
                            x4 = sb_xT[:, ht, :].rearrange("p (a b) -> p a b", b=STRIDE)[:, :, 0]
                            for i in range(4):
                                mm(ps[i][:], x4[:, 128 * i:128 * (i + 1)],
                                   sb_vw[:, ht, :], start=(ht == 0), stop=(ht == 7))
                        for i in range(4):
                            nc.vector.tensor_copy(
                                sb_Vs[:, i, :, 0:64],
                                ps[i][:].rearrange("p (h d) -> p h d", h=HPG))

                    def qk1_wave(w_sb, w_out, wave):
                        ps = [psS.tile([128, 512], f32, tag="sc", name="ps_qk1")
                              for _ in range(2)]
                        for ht in range(8):
                            for u in range(2):
                                mm(ps[u][:], w_sb[:, ht, 128:256],
                                   sb_xT[:, ht, 1024 * wave + 512 * u:
                                         1024 * wave + 512 * (u + 1)],
                                   start=(ht == 0), stop=(ht == 7))
                        for u in range(2):
                            nc.vector.tensor_copy(
                                w_out[:, 1, 1024 * wave + 512 * u:
                                      1024 * wave + 512 * (u + 1)], ps[u][:])

                    # PE fillers for phase A, in dependency order: the V
                    # projections, then ct=1 Q/K so head 3's scores can also
                    # run inside phase A (hiding its exps under PE work)
                    from collections import deque
                    fillers = deque([lambda g=g: v_group(g) for g in range(5)])
                    fillers.append(v_strided)
                    fillers.append(lambda: qk1_wave(sb_qw, sb_QT, 0))
                    fillers.append(lambda: qk1_wave(sb_qw, sb_QT, 1))
                    fillers.append(lambda: qk1_wave(sb_kw, sb_KT, 0))
                    fillers.append(lambda: (qk1_wave(sb_kw, sb_KT, 1),
                                            ks_compact(1)))

                    def fA():
                        if fillers:
                            fillers.popleft()()

                    def alloc_at():
                        ats = [p_ats.tile([128, S], DT, tag="ats", name="at_s")
                               for _ in range(4)]
                        atb = p_atb.tile([128, S], DT, tag="atb", name="at_b")
                        return ats, atb

                    at_s1, at_b1 = alloc_at()
                    emit_scores(1, at_s1, at_b1, fA)
                    at_s0, at_b0 = alloc_at()
                    emit_scores(0, at_s0, at_b0, fA)
                    while fillers:
                        fillers.popleft()()
                    at_s3, at_b3 = alloc_at()
                    emit_scores(3, at_s3, at_b3, lambda: None)

                # ---------------- Phase B ----------------
                if True:
                    # heads 1/0/3's attn@V blocks interleave into head 2's
                    # score waves; head 2's attn@V runs last (direct outTs
                    # write, shortest possible tail before phase C)
                    ost = p_ost.tile([64, 2, S], DT, tag="ost", name="ost")
                    avq = deque(av_blocks(1, at_s1, at_b1, ost)
                                + av_blocks(0, at_s0, at_b0, ost)
                                + av_blocks(3, at_s3, at_b3, ost))

                    def fB():
                        for _ in range(3):
                            if avq:
                                avq.popleft()()

                    at_s2, at_b2 = alloc_at()
                    emit_scores(2, at_s2, at_b2, fB)
                    while avq:
                        avq.popleft()()
                    for job in av_blocks(2, at_s2, at_b2, ost):
                        job()

                # ---------------- Phase C: F^T = (heads @ o_w)^T ----------------
                if True:
                    cp2 = 0
                    fT_r = d_fT.rearrange("(t p) s -> p t s", p=128)
                    st = None
                    for ft in range(8):
                        if ft % 2 == 0:
                            st = p_stage.tile([128, 2, S], OUT_DT, tag="st",
                                              name="stg")
                        for qs in range(4):
                            fpool = psS if cp2 % 3 == 2 else psB
                            ps = fpool.tile([128, 512], f32, tag="sc" if fpool is psS else "b", name="ps_ft")
                            for ctt in range(2):
                                mm(ps[:], sb_ow[:, ctt, 128 * ft:128 * (ft + 1)],
                                   sb_outTs[:, ctt, 512 * qs:512 * (qs + 1)],
                                   start=(ctt == 0), stop=(ctt == 1))
                            if cp2 % 2 == 0:
                                nc.vector.tensor_copy(
                                    st[:, ft % 2, 512 * qs:512 * (qs + 1)], ps[:])
                            else:
                                nc.scalar.copy(
                                    st[:, ft % 2, 512 * qs:512 * (qs + 1)], ps[:])
                            cp2 += 1
                        if ft % 2 == 1:
                            # issued on ACT so the SP sequencer reaches the
                            # next loop iteration's input DMA mid-phase-B
                            # (otherwise every iteration boundary stalls on
                            # the x/weights transfer)
                            nc.scalar.dma_start(out=fT_r[:, ft - 1:ft + 1, :],
                                                in_=st[:])

    nc.compile()
    return nc



def _free_bcast(ap, n):
    """Broadcast a [P, W] AP along a new middle free axis of length n."""
    import concourse.bass as bass
    return bass.AP(tensor=ap.tensor, offset=ap.offset,
                   ap=[list(ap.ap[0]), [0, n], list(ap.ap[1])])


def get_nc():
    key = (os.environ.get("KERNEL_MM_DTYPE", "bfloat16"),
           bool(os.environ.get("KERNEL_OUT_BF16")))
    if key not in _CACHE:
        _CACHE[key] = build_nc()
    return _CACHE[key]


def host_inputs(x, q_w, k_w, v_w, o_w, o_b, unity_scale):
    """Per-core input maps."""
    import ml_dtypes
    name = os.environ.get("KERNEL_MM_DTYPE", "bfloat16")
    np_dt = {"float32": np.float32, "float32r": np.float32,
             "bfloat16": ml_dtypes.bfloat16}[name]
    sig = 1.0 / (1.0 + np.exp(-float(np.asarray(unity_scale))))
    qw_eff = (np.asarray(q_w) * (sig / np.sqrt(D))).astype(np_dt)
    xT = np.ascontiguousarray(np.asarray(x).transpose(0, 2, 1)).astype(np_dt)
    maskb = host_masks().astype(np_dt)
    k_w = np.asarray(k_w).astype(np_dt)
    v_w = np.asarray(v_w).astype(np_dt)
    o_w = np.asarray(o_w).astype(np_dt)
    mask_rows = np.zeros((H, 2 * QCW), dtype=np_dt)
    mask_rows[0:128, :] = maskb
    in_maps = []
    for c in range(8):
        b, g = c // 4, c % 4
        cs = slice(GC * g, GC * (g + 1))
        xin = np.concatenate(
            [xT[b], qw_eff[:, cs], k_w[:, cs], v_w[:, cs], mask_rows], axis=1)
        in_maps.append({
            "xin": np.ascontiguousarray(xin),
            "ow": np.ascontiguousarray(o_w[cs, :]),
        })
    return in_maps


def kernel(x, q_w, k_w, v_w, o_w, o_b, unity_scale):
    global LAST_RESULTS
    from concourse.bass_utils import run_bass_kernel_spmd

    nc = get_nc()
    in_maps = host_inputs(x, q_w, k_w, v_w, o_w, o_b, unity_scale)
    res = run_bass_kernel_spmd(nc, in_maps, core_ids=list(range(8)),
                               trace=bool(os.environ.get("KERNEL_TRACE")))
    LAST_RESULTS = res
    out = np.zeros((B, S, H), np.float32)
    for b in range(B):
        acc = np.zeros((H, S), np.float32)
        for g in range(4):
            acc += np.asarray(res.results[4 * b + g]["fT"], np.float32)
        out[b] = acc.T
    out += np.asarray(o_b, np.float32)[None, None, :]
    return out



# revision 4
# speedup vs baseline: 1.2078x; 1.2078x over previous
"""Trainium2 Bass kernel for sparse (strided) multi-head attention.

Reference computation (B=2, S=2048, H=1024, NH=16, D=64):
    q = (x @ q_w) * sigmoid(phi); k = x @ k_w; v = x @ v_w   (per-head [S, D])
    scores = q k^T / sqrt(D), masked to allowed[i, j] = (j % 4 == 0) | (|i-j| <= 8)
    out = softmax(scores) @ v;  return concat_heads(out) @ o_w + o_b

Sharding: 8 cores = 2 batches x 4 head-groups (4 heads each). Each core gets
x^T for its batch, column-sliced q/k/v weights, row-sliced o_w, and returns a
partial transposed output F^T = (attn_out_heads @ o_w_slice)^T which the host
sums over head-groups, transposes, and biases.

v3 design notes (all matmuls bf16; PSUM stays f32):
  - Band handling as in v2: 19 query-chunks of 112 queries against 128-key
    windows starting 8 keys early; multiplicative 0/1 post-exp masks.
  - attn@V runs with QUERIES on the psum partition axis: per 128-query
    subtile, lhsT = exp'd scores [128 keys, 128 q] (full-width M) and
    rhs = V[keys, 65] (64 values + a ones column). This halves the PE cost
    vs the v2 value-major layout (whose M=65 wasted half the array) and the
    ones column lands the softmax denominator as a per-partition scalar:
    normalize is one DVE copy + tiny reciprocal + one Pool mul with a
    free-axis broadcast -- no partition_broadcast, no [1,S] reciprocals,
    no repartition DMA.
  - PE transposes (via a shipped 128x128 identity) restore the c-major
    [channels, queries] layout the output projection needs.
  - The output projection runs PER 512-QUERY BLOCK as soon as all four
    heads' normalized output for that block is transposed, overlapping the
    former serial phase C tail with the attention tail; each block ships in
    one DMA issued from ACT.
  - Input DMAs are split (weights / masks+identity / 4 x^T column chunks)
    so the first QK projections start as soon as the first x^T chunk lands
    and cross-iteration reuse hazards are per-piece.
  - Engine budget (sim): PE ~95us (projections + scores + attn@V + out
    proj), ACT ~53us (exps + 1/4 of out-proj psum copies), DVE ~47us
    (psum copies, small reciprocals), Pool ~23us (0/1 masks, normalize).
"""

import os
import numpy as np

B, S, H = 2, 2048, 1024
NH, D = 16, 64
PHI = 1.6180339887
STRIDE, LOCAL = 4, 8
HPG = 4              # heads per group (= per core)
GC = HPG * D         # channels per core = 256
NSK = S // STRIDE    # 512 strided keys
QCW = 64             # band query-chunk width (64+2*8 = 80-key windows)
NQC = 32             # number of band chunks (32*64 = 2048, no tail case)
XW = S + 3 * GC + 2 * QCW + 128  # x^T | q/k/v weights | masks | identity

_CACHE = {}
LAST_RESULTS = None  # BassKernelResults of the most recent run (for profiling)


def _chunk_geom(c):
    """(key-window start, window width, query base, query count) for chunk c."""
    w0 = max(0, QCW * c - LOCAL)
    w1 = min(S, QCW * c + QCW + LOCAL)
    return w0, w1 - w0, QCW * c, QCW


def _band_pieces(qs0):
    """Band chunks overlapping query subtile [qs0, qs0+128): (c, kw, lo, hi).

    With QCW=64 these are exactly two chunks at psum offsets 0 and 64 (the
    only base partitions PE matmul output accepts besides 32)."""
    c0 = qs0 // QCW
    return [(c, _chunk_geom(c)[1], QCW * c, QCW * c + QCW)
            for c in (c0, c0 + 1)]


def host_masks():
    """Multiplicative 0/1 masks applied to exp'd band scores.

    mask0 is for the key-aligned chunk 0 (key kp, query q):
        |q - kp| <= 8 and kp % 4 != 0
    maskN is for the shifted chunks c >= 1 (key 112c-8+kp):
        kp-16 <= q <= kp and kp % 4 != 0
    """
    kp = np.arange(128)[:, None]
    q = np.arange(QCW)[None, :]
    mask0 = ((np.abs(q - kp) <= LOCAL) & (kp % STRIDE != 0)).astype(np.float32)
    maskN = ((q >= kp - 2 * LOCAL) & (q <= kp) & (kp % STRIDE != 0)).astype(np.float32)
    return np.concatenate([mask0, maskN], axis=1)  # [128, 128]


def _dtypes():
    import concourse.mybir as mybir
    name = os.environ.get("KERNEL_MM_DTYPE", "bfloat16")
    dt = {"float32": mybir.dt.float32, "float32r": mybir.dt.float32r,
          "bfloat16": mybir.dt.bfloat16}[name]
    out_dt = (mybir.dt.float32 if os.environ.get("KERNEL_OUT_F32")
              else mybir.dt.bfloat16)
    return dt, out_dt


def build_nc(loop_n=1, unroll=False):
    """Build the per-core Bass program (same NEFF for all 8 cores).

    loop_n > 1 wraps the whole pipeline in a hardware loop (benchmarking:
    wall-clock deltas between loop counts cancel dispatch overhead).
    unroll=True python-unrolls instead (for TimelineSim, which cannot
    resolve For_i branches).

    Emission order is engine program order, so the stream is scheduled
    explicitly for overlap:
      Phase A: K ct0 per x-chunk as it lands, then head 1 scores with
        Q ct0 / QK ct1 waves as PE fillers, head 0 scores with the V
        projections as fillers -- ACT chews exps under projection work.
      Phase B: head 3 scores with head 1's attn@V blocks as fillers, head
        2 scores with heads 0/3's attn@V; then per 512-query block: head
        2's attn@V, the 8 PE transposes, and that block's output
        projection + DMA. The out-proj of block b overlaps attn of b+1.
    """
    import contextlib
    import concourse.mybir as mybir
    import concourse.tile as tile
    from concourse import bacc
    from collections import deque

    f32 = mybir.dt.float32
    DT, OUT_DT = _dtypes()
    AF = mybir.ActivationFunctionType

    nc = bacc.Bacc("TRN2", target_bir_lowering=False, debug=False)

    d_xin = nc.dram_tensor("xin", [H, XW], DT, kind="ExternalInput")
    d_ow = nc.dram_tensor("ow", [GC, H], DT, kind="ExternalInput")
    d_fT = nc.dram_tensor("fT", [H, S], OUT_DT, kind="ExternalOutput")

    def mm(out, lhsT, rhs, start, stop):
        nc.tensor.matmul(out, lhsT, rhs, start=start, stop=stop,
                         skip_group_check=True)

    with tile.TileContext(nc) as tc:
        with (
            tc.tile_pool(name="persist", bufs=1) as persist,
            tc.tile_pool(name="ph1", bufs=1) as ph1,
        ):
            sb_ow = persist.tile([128, 2, 1024], DT)

            # D-major Q^T / K^T: [128ch (2 heads), c-tile, S]
            sb_QT = persist.tile([128, 2, S], DT)
            sb_KT = persist.tile([128, 2, S], DT)
            sb_KsT = persist.tile([128, 2, NSK], DT)      # strided keys, compacted
            # S-major V, shifted band windows + strided keys; col 64 = 1.0
            sb_Vsh = persist.tile([128, NQC, HPG, 66], DT)  # 32 windows
            sb_Vs = persist.tile([128, NSK // 128, HPG, 66], DT)
            sb_outTs = persist.tile([128, 2, S], DT)      # c-major head outputs
            # normalized attn out, q-major: [q, block, subtile, 256 ch]
            sb_stn = persist.tile([128, 4, 4, 256], DT)

            sb_xin = ph1.tile([128, 8, XW], DT)
            sb_xT = sb_xin[:, :, 0:S]
            sb_qw = sb_xin[:, :, S:S + GC]
            sb_kw = sb_xin[:, :, S + GC:S + 2 * GC]
            sb_vw = sb_xin[:, :, S + 2 * GC:S + 3 * GC]
            sb_maskb = sb_xin[:, 0, S + 3 * GC:S + 3 * GC + 2 * QCW].rearrange(
                "p (m q) -> p m q", m=2)
            sb_ident = sb_xin[:, 0, S + 3 * GC + 2 * QCW:XW]

            loop_cm = (tc.For_i(0, loop_n, 1) if loop_n > 1 and not unroll
                       else contextlib.nullcontext())
            with loop_cm, (
                tc.tile_pool(name="ats", bufs=12)) as p_ats, (
                tc.tile_pool(name="atb", bufs=4)) as p_atb, (
                tc.tile_pool(name="stg", bufs=3)) as p_stg, (
                tc.tile_pool(name="rec", bufs=3)) as p_rec, (
                tc.tile_pool(name="stC", bufs=2)) as p_stC, (
                tc.tile_pool(name="psS", bufs=2, space="PSUM")) as psS, (
                tc.tile_pool(name="psB", bufs=4, space="PSUM")) as psB, (
                tc.tile_pool(name="psC", bufs=2, space="PSUM")) as psC:

              for _it in range(loop_n if unroll else 1):
                # ---------------- shared emit helpers ----------------
                def hslices(h):
                    ct, pb = h // 2, (h % 2) * 64
                    return (ct, pb, sb_QT[pb:pb + 64, ct, :],
                            sb_KT[pb:pb + 64, ct, :], sb_KsT[pb:pb + 64, ct, :])

                def sc_strided(h, b, at_s):
                    """Strided scores for query block b (all 4 key tiles)."""
                    ct, pb, QT, KT, KsT = hslices(h)
                    ql = slice(512 * b, 512 * (b + 1))
                    for i in range(4):
                        ps = psS.tile([128, 512], f32, tag="sc", name="ps_sc")
                        mm(ps[:], KsT[:, 128 * i:128 * (i + 1)], QT[:, ql],
                           start=True, stop=True)
                        nc.scalar.activation(at_s[i][:, ql], ps[:], AF.Exp)

                def band_group(h, g, at_b):
                    """Band-score psum group g (8 chunks): mm + exp + mask.

                    Rows kw:128 of each chunk's columns are stale psum exp'd
                    to garbage; the masks zero rows [0:80] that are disallowed
                    and attn@V reads only rows [0:kw], so garbage never flows.
                    """
                    ct, pb, QT, KT, _ = hslices(h)
                    ps = psS.tile([128, 512], f32, tag="sc", name="ps_bd")
                    for j, c in enumerate(range(8 * g, 8 * g + 8)):
                        w0, kw_, qb, nq = _chunk_geom(c)
                        mm(ps[0:kw_, 64 * j:64 * j + nq],
                           KT[:, w0:w0 + kw_], QT[:, qb:qb + nq],
                           start=True, stop=True)
                    nc.scalar.activation(at_b[:, 512 * g:512 * (g + 1)],
                                         ps[:], AF.Exp)
                    mN = sb_maskb[0:80, 1, :]
                    if g == 0:
                        nc.gpsimd.tensor_mul(at_b[0:80, 0:64], at_b[0:80, 0:64],
                                             sb_maskb[0:80, 0, :])
                        sl = at_b[0:80, 64:512].rearrange("p (a b) -> p a b", b=64)
                        nc.gpsimd.tensor_mul(sl, sl, _free_bcast(mN, 7))
                    else:
                        sl = at_b[0:80, 512 * g:512 * (g + 1)].rearrange(
                            "p (a b) -> p a b", b=64)
                        nc.gpsimd.tensor_mul(sl, sl, _free_bcast(mN, 8))

                def emit_scores(h, at_s, at_b, filler):
                    """All scores for head h; filler() emits PE work between
                    waves."""
                    for b in range(4):
                        sc_strided(h, b, at_s)
                        filler()
                    for g in range(4):
                        band_group(h, g, at_b)
                        if g % 2 == 1:
                            filler()

                def av_block(h, b, at_s, at_b):
                    """attn@[V|1] for head h, query block b, q on partitions.

                    Per 128-q subtile g the psum columns [65g, 65g+65) hold
                    values 0:64 and the softmax denominator at 64. One DVE
                    copy drains the block, a [128,4] reciprocal and one Pool
                    mul (free-axis broadcast) write the normalized output
                    into sb_stn's channel slice for this head.
                    """
                    av = psB.tile([128, 512], f32, tag="b", name="ps_av")
                    for g in range(4):
                        qs0 = 512 * b + 128 * g
                        col = slice(65 * g, 65 * g + 65)
                        for i in range(4):
                            mm(av[:, col], at_s[i][:, qs0:qs0 + 128],
                               sb_Vs[:, i, h, 0:65],
                               start=(i == 0), stop=False)
                        pieces = _band_pieces(qs0)
                        for j, (c, kw_, lo, hi) in enumerate(pieces):
                            mm(av[lo - qs0:hi - qs0, col],
                               at_b[0:kw_, lo:hi], sb_Vsh[0:kw_, c, h, 0:65],
                               start=False, stop=(j == len(pieces) - 1))
                    st = p_stg.tile([128, 4, 65], DT, tag="stg", name="st_av")
                    nc.vector.tensor_copy(
                        st[:], av[:, 0:260].rearrange("p (g c) -> p g c", c=65))
                    rec = p_rec.tile([128, 4], DT, tag="rec", name="rec")
                    with nc.allow_low_precision("bf16 softmax denominators"):
                        nc.vector.reciprocal(rec[:], st[:, :, 64])
                    nc.gpsimd.tensor_mul(sb_stn[:, b, :, 64 * h:64 * h + 64],
                                         st[:, :, 0:64],
                                         _free_bcast_last(rec[:], 64))

                def tp_block(b):
                    """Transpose block b's normalized [q, ch] to c-major."""
                    for c in range(2):
                        # transpose is pass-through: psum holds bf16 here
                        tp = psB.tile([128, 512], DT, tag="b", name="ps_tp")
                        for g in range(4):
                            nc.tensor.transpose(
                                tp[:, 128 * g:128 * (g + 1)],
                                sb_stn[:, b, g, 128 * c:128 * (c + 1)],
                                sb_ident)
                        nc.vector.tensor_copy(
                            sb_outTs[:, c, 512 * b:512 * (b + 1)], tp[:])

                def out_block(b, fT_r):
                    """Output projection + DMA for query block b."""
                    stC = p_stC.tile([128, 8, 512], OUT_DT, tag="stC", name="stC")
                    for ft in range(8):
                        pc = psC.tile([128, 512], f32, tag="c", name="ps_ft")
                        for ctt in range(2):
                            mm(pc[:], sb_ow[:, ctt, 128 * ft:128 * (ft + 1)],
                               sb_outTs[:, ctt, 512 * b:512 * (b + 1)],
                               start=(ctt == 0), stop=(ctt == 1))
                        if ft % 4 == 3:
                            nc.scalar.copy(stC[:, ft, :], pc[:])
                        else:
                            nc.vector.tensor_copy(stC[:, ft, :], pc[:])
                    # issued on ACT so the SP sequencer reaches the next loop
                    # iteration's input DMAs mid-iteration
                    nc.scalar.dma_start(out=fT_r[:, :, 512 * b:512 * (b + 1)],
                                        in_=stC[:])

                # ---------------- Phase A ----------------
                if True:
                    xin_r = d_xin.rearrange("(t p) s -> p t s", p=128)
                    fT_r = d_fT.rearrange("(t p) s -> p t s", p=128)
                    # weights+masks+identity first, then x^T column chunks in
                    # consumption order; each is a separate transfer so the
                    # first QK wave starts as soon as chunk 0 lands
                    nc.sync.dma_start(out=sb_xin[:, :, S:XW], in_=xin_r[:, :, S:XW])
                    for i in range(4):
                        nc.sync.dma_start(out=sb_xin[:, :, 512 * i:512 * (i + 1)],
                                          in_=xin_r[:, :, 512 * i:512 * (i + 1)])
                    nc.sync.dma_start(out=sb_ow[:], in_=d_ow.rearrange("(t p) f -> p t f", p=128))

                    # ones columns for the attn@V row-sum trick
                    nc.gpsimd.memset(sb_Vsh[:, :, :, 64], 1.0)
                    nc.gpsimd.memset(sb_Vs[:, :, :, 64], 1.0)

                    def qk0_part(w_sb, w_out, ss):
                        ps = psB.tile([128, 512], f32, tag="b", name="psproj")
                        for ht in range(8):
                            mm(ps[:], w_sb[:, ht, 0:128],
                               sb_xT[:, ht, 512 * ss:512 * (ss + 1)],
                               start=(ht == 0), stop=(ht == 7))
                        nc.vector.tensor_copy(
                            w_out[:, 0, 512 * ss:512 * (ss + 1)], ps[:])

                    def ks_compact(ct):
                        ks = sb_KT[:, ct, :].rearrange("p (r f) -> p r f", f=STRIDE)[:, :, 0]
                        nc.vector.tensor_copy(sb_KsT[:, ct, :], ks)

                    # V projection fillers: 5 shifted-window groups + strided
                    def v_group(g):
                        chunks = range(4 * g, 4 * g + 4)
                        ps = [psB.tile([128, GC], f32, tag="b", name="psprojv")
                              for _ in chunks]
                        for ht in range(8):
                            for j, c in enumerate(chunks):
                                w0, kw_, _, _ = _chunk_geom(c)
                                mm(ps[j][0:kw_, :], sb_xT[:, ht, w0:w0 + kw_],
                                   sb_vw[:, ht, :], start=(ht == 0), stop=(ht == 7))
                        for j, c in enumerate(chunks):
                            _, kw_, _, _ = _chunk_geom(c)
                            nc.vector.tensor_copy(
                                sb_Vsh[0:kw_, c, :, 0:64],
                                ps[j][0:kw_, :].rearrange("p (h d) -> p h d", h=HPG))

                    def v_strided():
                        ps = [psB.tile([128, GC], f32, tag="b", name="psprojs")
                              for _ in range(4)]
                        for ht in range(8):
                            x4 = sb_xT[:, ht, :].rearrange("p (a b) -> p a b", b=STRIDE)[:, :, 0]
                            for i in range(4):
                                mm(ps[i][:], x4[:, 128 * i:128 * (i + 1)],
                                   sb_vw[:, ht, :], start=(ht == 0), stop=(ht == 7))
                        for i in range(4):
                            nc.vector.tensor_copy(
                                sb_Vs[:, i, :, 0:64],
                                ps[i][:].rearrange("p (h d) -> p h d", h=HPG))

                    def qk1_wave(w_sb, w_out, wave):
                        ps = [psS.tile([128, 512], f32, tag="sc", name="ps_qk1")
                              for _ in range(2)]
                        for ht in range(8):
                            for u in range(2):
                                mm(ps[u][:], w_sb[:, ht, 128:256],
                                   sb_xT[:, ht, 1024 * wave + 512 * u:
                                         1024 * wave + 512 * (u + 1)],
                                   start=(ht == 0), stop=(ht == 7))
                        for u in range(2):
                            nc.vector.tensor_copy(
                                w_out[:, 1, 1024 * wave + 512 * u:
                                      1024 * wave + 512 * (u + 1)], ps[u][:])

                    # K ct0 as chunks land, then head 1 scores can start
                    for ss in range(4):
                        qk0_part(sb_kw, sb_KT, ss)
                    ks_compact(0)
                    qk0_part(sb_qw, sb_QT, 0)

                    def alloc_at():
                        ats = [p_ats.tile([128, S], DT, tag="ats", name="at_s")
                               for _ in range(4)]
                        atb = p_atb.tile([128, S], DT, tag="atb", name="at_b")
                        return ats, atb

                    fillers = deque()
                    fillers.append(lambda: qk0_part(sb_qw, sb_QT, 1))
                    fillers.append(lambda: qk0_part(sb_qw, sb_QT, 2))
                    fillers.append(lambda: qk0_part(sb_qw, sb_QT, 3))
                    fillers.append(lambda: qk1_wave(sb_qw, sb_QT, 0))
                    fillers.append(lambda: qk1_wave(sb_qw, sb_QT, 1))
                    fillers.append(lambda: qk1_wave(sb_kw, sb_KT, 0))
                    fillers.append(lambda: (qk1_wave(sb_kw, sb_KT, 1),
                                            ks_compact(1)))

                    def fA():
                        if fillers:
                            fillers.popleft()()

                    at_s1, at_b1 = alloc_at()
                    emit_scores(1, at_s1, at_b1, fA)
                    fillers.extend([lambda g=g: v_group(g) for g in range(8)])
                    fillers.append(v_strided)
                    at_s0, at_b0 = alloc_at()
                    emit_scores(0, at_s0, at_b0, fA)
                    while fillers:
                        fillers.popleft()()

                # ---------------- Phase B ----------------
                if True:
                    # heads 1/0/3's attn@V blocks fill heads 3/2's score
                    # waves; then per query block: head 2's attn@V, the
                    # transposes, and the block's output projection
                    at_s3, at_b3 = alloc_at()
                    avq = deque([lambda b=b: av_block(1, b, at_s1, at_b1)
                                 for b in range(4)])

                    def fB():
                        if avq:
                            avq.popleft()()

                    emit_scores(3, at_s3, at_b3, fB)
                    avq.extend([lambda b=b: av_block(0, b, at_s0, at_b0)
                                for b in range(4)])
                    avq.extend([lambda b=b: av_block(3, b, at_s3, at_b3)
                                for b in range(3)])
                    at_s2, at_b2 = alloc_at()
                    emit_scores(2, at_s2, at_b2, fB)
                    while avq:
                        avq.popleft()()
                    av_block(3, 3, at_s3, at_b3)

                    av_block(2, 0, at_s2, at_b2)
                    av_block(2, 1, at_s2, at_b2)
                    tp_block(0)
                    out_block(0, fT_r)
                    av_block(2, 2, at_s2, at_b2)
                    tp_block(1)
                    out_block(1, fT_r)
                    av_block(2, 3, at_s2, at_b2)
                    tp_block(2)
                    out_block(2, fT_r)
                    tp_block(3)
                    out_block(3, fT_r)

    nc.compile()
    return nc


def _free_bcast(ap, n):
    """Broadcast a [P, W] AP along a new middle free axis of length n."""
    import concourse.bass as bass
    return bass.AP(tensor=ap.tensor, offset=ap.offset,
                   ap=[list(ap.ap[0]), [0, n], list(ap.ap[1])])


def _free_bcast_last(ap, n):
    """Broadcast a [P, W] AP along a new last free axis of length n."""
    import concourse.bass as bass
    return bass.AP(tensor=ap.tensor, offset=ap.offset,
                   ap=[list(ap.ap[0]), list(ap.ap[1]), [0, n]])


def get_nc():
    key = (os.environ.get("KERNEL_MM_DTYPE", "bfloat16"),
           bool(os.environ.get("KERNEL_OUT_F32")))
    if key not in _CACHE:
        _CACHE[key] = build_nc()
    return _CACHE[key]


def host_inputs(x, q_w, k_w, v_w, o_w, o_b, unity_scale):
    """Per-core input maps."""
    import ml_dtypes
    name = os.environ.get("KERNEL_MM_DTYPE", "bfloat16")
    np_dt = {"float32": np.float32, "float32r": np.float32,
             "bfloat16": ml_dtypes.bfloat16}[name]
    sig = 1.0 / (1.0 + np.exp(-float(np.asarray(unity_scale))))
    qw_eff = (np.asarray(q_w) * (sig / np.sqrt(D))).astype(np_dt)
    xT = np.ascontiguousarray(np.asarray(x).transpose(0, 2, 1)).astype(np_dt)
    maskb = host_masks().astype(np_dt)
    k_w = np.asarray(k_w).astype(np_dt)
    v_w = np.asarray(v_w).astype(np_dt)
    o_w = np.asarray(o_w).astype(np_dt)
    tail_rows = np.zeros((H, 2 * QCW + 128), dtype=np_dt)
    tail_rows[0:128, 0:2 * QCW] = maskb
    tail_rows[0:128, 2 * QCW:] = np.eye(128, dtype=np_dt)
    in_maps = []
    for c in range(8):
        b, g = c // 4, c % 4
        cs = slice(GC * g, GC * (g + 1))
        xin = np.concatenate(
            [xT[b], qw_eff[:, cs], k_w[:, cs], v_w[:, cs], tail_rows], axis=1)
        in_maps.append({
            "xin": np.ascontiguousarray(xin),
            "ow": np.ascontiguousarray(o_w[cs, :]),
        })
    return in_maps


def kernel(x, q_w, k_w, v_w, o_w, o_b, unity_scale):
    global LAST_RESULTS
    from concourse.bass_utils import run_bass_kernel_spmd

    nc = get_nc()
    in_maps = host_inputs(x, q_w, k_w, v_w, o_w, o_b, unity_scale)
    res = run_bass_kernel_spmd(nc, in_maps, core_ids=list(range(8)),
                               trace=bool(os.environ.get("KERNEL_TRACE")))
    LAST_RESULTS = res
    out = np.zeros((B, S, H), np.float32)
    for b in range(B):
        acc = np.zeros((H, S), np.float32)
        for g in range(4):
            acc += np.asarray(res.results[4 * b + g]["fT"], np.float32)
        out[b] = acc.T
    out += np.asarray(o_b, np.float32)[None, None, :]
    return out
